# revision 2
# baseline (speedup 1.0000x reference)
"""CLIP contrastive loss on 8 Trainium2 NeuronCores.

Strategy (data parallel over rows, per the sharding hint):
  - Both feature matrices are row-sharded 8 x [2048, 512].
  - Each core PE-transposes its shards to D-major (cast to bf16), then
    AllGathers both transposed matrices (bf16 halves the wire bytes).
  - Pass 1 (image rows): L = img_shard @ txt_all^T computed in [128, 2048]
    PSUM tiles; per tile: VectorE rowmax, then one fused ScalarE
    activation exp(SCALE*x - SCALE*max) with free-dim sum accumulation.
    Per-row-block group stats are merged into a logsumexp at the end.
  - Pass 2 (text rows): same with roles swapped.
  - diag terms <img_i, txt_i> computed in fp32 on VectorE.
  - Each core outputs [128, 4] partial sums; the host reduces them to the
    scalar loss. All O(N^2) work happens on-device.
"""
import sys

if "/opt/trn_rl_repo" not in sys.path:
    sys.path.insert(0, "/opt/trn_rl_repo")

import numpy as np

from concourse import bacc, mybir, tile
from concourse.bass_utils import run_bass_kernel_spmd
from concourse.masks import make_identity

SCALE = 1.0 / 0.07
N = 16384
D = 512
NCORES = 8
LN = N // NCORES          # 2048 local rows
P = 128
R = LN // P               # 16 row tiles per core
KC = D // P               # 4 contraction chunks
CH = 512                  # matmul moving free dim (one PSUM bank)
CPB = LN // CH            # 4 chunks per block
NB = NCORES               # 8 column blocks (one per source core)

F32 = mybir.dt.float32
BF16 = mybir.dt.bfloat16


def build():
    nc = bacc.Bacc(None, target_bir_lowering=False, debug=False, num_devices=NCORES)

    img_ext = nc.dram_tensor("image_features", [LN, D], F32, kind="ExternalInput")
    txt_ext = nc.dram_tensor("text_features", [LN, D], F32, kind="ExternalInput")
    out_ext = nc.dram_tensor("out", [P, 4], F32, kind="ExternalOutput")

    with tile.TileContext(nc) as tc:
        with (
            tc.tile_pool(name="dram", bufs=1, space="DRAM") as dram,
            tc.tile_pool(name="const", bufs=1) as const,
            tc.tile_pool(name="persist", bufs=1) as persist,
            tc.tile_pool(name="stats", bufs=1) as stats,
        ):
            itb = dram.tile([D, LN], BF16)
            ttb = dram.tile([D, LN], BF16)
            itg = dram.tile([NCORES * D, LN], BF16, addr_space="Shared")
            ttg = dram.tile([NCORES * D, LN], BF16, addr_space="Shared")

            ident = const.tile([P, P], F32)
            make_identity(nc, ident)

            # persistent D-major bf16 shards: [p=d%128, dk, i]
            imgT = persist.tile([P, KC, LN], BF16)
            txtT = persist.tile([P, KC, LN], BF16)

            # per (row-tile, block) group stats, col index = r * NB + b
            mx0 = stats.tile([P, R * NB], F32)
            ss0 = stats.tile([P, R * NB], F32)
            mx1 = stats.tile([P, R * NB], F32)
            ss1 = stats.tile([P, R * NB], F32)
            diag_pp = stats.tile([P, 1], F32)
            lse0_pp = stats.tile([P, 1], F32)
            lse1_pp = stats.tile([P, 1], F32)

            # ---------------- setup: load, diag, transpose, gather ----------
            with (
                tc.tile_pool(name="setup", bufs=1) as setup,
                tc.tile_pool(name="tpsum", bufs=2, space="PSUM") as tpsum,
            ):
                img_sb = setup.tile([P, R, D], F32)
                txt_sb = setup.tile([P, R, D], F32)
                nc.sync.dma_start(img_sb[:], img_ext[:].rearrange("(r p) d -> p r d", p=P))
                nc.sync.dma_start(txt_sb[:], txt_ext[:].rearrange("(r p) d -> p r d", p=P))

                # text first so its AllGather is issued as early as possible
                for src, dstT in ((txt_sb, txtT), (img_sb, imgT)):
                    for r in range(R):
                        tp = tpsum.tile([P, KC, P], F32, name="tp")
                        for dk in range(KC):
                            nc.tensor.transpose(
                                tp[:, dk, :],
                                src[:, r, dk * P:(dk + 1) * P],
                                ident[:],
                            )
                        nc.vector.tensor_copy(
                            dstT[:, :, r * P:(r + 1) * P], tp[:]
                        )
                    if dstT is txtT:
                        nc.sync.dma_start(
                            ttb[:].rearrange("(dk p) i -> p dk i", p=P), txtT[:]
                        )
                        nc.gpsimd.collective_compute(
                            "AllGather",
                            mybir.AluOpType.bypass,
                            replica_groups=[list(range(NCORES))],
                            ins=[ttb[:].opt()],
                            outs=[ttg[:].opt()],
                        )
                    else:
                        nc.sync.dma_start(
                            itb[:].rearrange("(dk p) i -> p dk i", p=P), imgT[:]
                        )
                        nc.gpsimd.collective_compute(
                            "AllGather",
                            mybir.AluOpType.bypass,
                            replica_groups=[list(range(NCORES))],
                            ins=[itb[:].opt()],
                            outs=[itg[:].opt()],
                        )

                # diag partial: sum_d img[i, d] * txt[i, d]  (unscaled, fp32)
                dtmp = setup.tile([P, R, D], F32)
                nc.vector.tensor_mul(dtmp[:], img_sb[:], txt_sb[:])
                dsum = setup.tile([P, R], F32)
                nc.vector.reduce_sum(dsum[:], dtmp[:], axis=mybir.AxisListType.X)
                nc.vector.reduce_sum(diag_pp[:], dsum[:], axis=mybir.AxisListType.X)

            # ---------------- main passes ----------------------------------
            with (
                tc.tile_pool(name="stream", bufs=2) as stream,
                tc.tile_pool(name="mpsum", bufs=2, space="PSUM") as mpsum,
                tc.tile_pool(name="scratch", bufs=2) as scratch,
                tc.tile_pool(name="small", bufs=4) as small,
            ):
                for lhsT, gsrc, mx, ss, lse_pp in (
                    (imgT, ttg, mx0, ss0, lse0_pp),
                    (txtT, itg, mx1, ss1, lse1_pp),
                ):
                    for b in range(NB):
                        rhs = stream.tile([P, KC, LN], BF16, name="rhs", tag="rhs")
                        nc.sync.dma_start(
                            rhs[:],
                            gsrc[b * D:(b + 1) * D, :].rearrange(
                                "(dk p) j -> p dk j", p=P
                            ),
                        )
                        for r in range(R):
                            pt = mpsum.tile([P, CPB * CH], F32, name="pt", tag="pt")
                            for c in range(CPB):
                                for k in range(KC):
                                    nc.tensor.matmul(
                                        pt[:, c * CH:(c + 1) * CH],
                                        lhsT[:, k, r * P:(r + 1) * P],
                                        rhs[:, k, c * CH:(c + 1) * CH],
                                        start=(k == 0),
                                        stop=(k == KC - 1),
                                    )
                            col = r * NB + b
                            nc.vector.reduce_max(
                                mx[:, col:col + 1], pt[:], axis=mybir.AxisListType.X
                            )
                            bias = small.tile([P, 1], F32, name="bias", tag="bias")
                            nc.vector.tensor_scalar_mul(
                                bias[:], mx[:, col:col + 1], -SCALE
                            )
                            esc = scratch.tile(
                                [P, CPB * CH], BF16, name="esc", tag="esc"
                            )
                            nc.scalar.activation(
                                esc[:],
                                pt[:],
                                mybir.ActivationFunctionType.Exp,
                                bias=bias[:],
                                scale=SCALE,
                                accum_out=ss[:, col:col + 1],
                            )

                    # epilogue: merge the NB group stats per row tile
                    nc.vector.memset(lse_pp[:], 0.0)
                    for r in range(R):
                        sl = slice(r * NB, (r + 1) * NB)
                        mh = small.tile([P, 1], F32, name="mh", tag="mh")
                        nc.vector.reduce_max(mh[:], mx[:, sl], axis=mybir.AxisListType.X)
                        dev = small.tile([P, NB], F32, name="dev", tag="dev")
                        nc.vector.tensor_scalar_sub(dev[:], mx[:, sl], mh[:])
                        w = small.tile([P, NB], F32, name="w", tag="w")
                        nc.scalar.activation(
                            w[:], dev[:], mybir.ActivationFunctionType.Exp, scale=SCALE
                        )
                        sw = small.tile([P, NB], F32, name="sw", tag="sw")
                        nc.vector.tensor_mul(sw[:], ss[:, sl], w[:])
                        stot = small.tile([P, 1], F32, name="stot", tag="stot")
                        nc.vector.reduce_sum(stot[:], sw[:], axis=mybir.AxisListType.X)
                        lgs = small.tile([P, 1], F32, name="lgs", tag="lgs")
                        nc.scalar.activation(
                            lgs[:], stot[:], mybir.ActivationFunctionType.Ln
                        )
                        # lse_r = SCALE * mh + log(s);  lse_pp += lse_r
                        lser = small.tile([P, 1], F32, name="lser", tag="lser")
                        nc.vector.scalar_tensor_tensor(
                            lser[:],
                            mh[:],
                            SCALE,
                            lgs[:],
                            op0=mybir.AluOpType.mult,
                            op1=mybir.AluOpType.add,
                        )
                        nc.vector.tensor_add(lse_pp[:], lse_pp[:], lser[:])

                out_sb = small.tile([P, 4], F32, name="out_sb", tag="out_sb")
                nc.vector.memset(out_sb[:], 0.0)
                nc.vector.tensor_copy(out_sb[:, 0:1], lse0_pp[:])
                nc.vector.tensor_copy(out_sb[:, 1:2], lse1_pp[:])
                nc.vector.tensor_copy(out_sb[:, 2:3], diag_pp[:])
                nc.sync.dma_start(out_ext[:], out_sb[:])

    nc.compile()
    return nc


_NC_CACHE = None


def _get_nc():
    global _NC_CACHE
    if _NC_CACHE is None:
        _NC_CACHE = build()
    return _NC_CACHE


def kernel(image_features: np.ndarray, text_features: np.ndarray) -> np.ndarray:
    img = np.ascontiguousarray(np.asarray(image_features, dtype=np.float32))
    txt = np.ascontiguousarray(np.asarray(text_features, dtype=np.float32))
    assert img.shape == (N, D) and txt.shape == (N, D)

    nc = _get_nc()
    in_maps = [
        {
            "image_features": img[i * LN:(i + 1) * LN],
            "text_features": txt[i * LN:(i + 1) * LN],
        }
        for i in range(NCORES)
    ]
    res = run_bass_kernel_spmd(nc, in_maps, core_ids=list(range(NCORES)))

    lse_img = 0.0
    lse_txt = 0.0
    diag = 0.0
    for om in res.results:
        part = om["out"].astype(np.float64)
        lse_img += part[:, 0].sum()
        lse_txt += part[:, 1].sum()
        diag += part[:, 2].sum()

    loss = (lse_img + lse_txt - 2.0 * SCALE * diag) / (2.0 * N)
    return np.float32(loss)


if __name__ == "__main__":
    rng = np.random.default_rng(0)
    a = rng.standard_normal((N, D)).astype(np.float32)
    b = rng.standard_normal((N, D)).astype(np.float32)
    print("loss:", kernel(a, b))


# revision 7
# speedup vs baseline: 1.2647x; 1.2647x over previous
"""CLIP contrastive loss on 8 Trainium2 NeuronCores.

Strategy (data parallel over rows, per the sharding hint):
  - Both feature matrices are row-sharded 8 x [2048, 512].
  - Each core PE-transposes its shards to D-major (cast to bf16), then
    AllGathers both transposed matrices (bf16 halves the wire bytes).
  - Pass 1 (image rows): L = img_shard @ txt_all^T computed in [128, 2048]
    PSUM tiles; per tile: VectorE rowmax, then one fused ScalarE
    activation exp(SCALE*x - SCALE*max) with free-dim sum accumulation.
    Per-row-block group stats are merged into a logsumexp at the end.
  - Pass 2 (text rows): same with roles swapped.
  - diag terms <img_i, txt_i> computed in fp32 on VectorE.
  - Each core outputs [128, 4] partial sums; the host reduces them to the
    scalar loss. All O(N^2) work happens on-device.
"""
import sys

if "/opt/trn_rl_repo" not in sys.path:
    sys.path.insert(0, "/opt/trn_rl_repo")

import numpy as np

from concourse import bacc, mybir, tile
from concourse.bass_utils import run_bass_kernel_spmd
from concourse.masks import make_identity

SCALE = 1.0 / 0.07
N = 16384
D = 512
NCORES = 8
LN = N // NCORES          # 2048 local rows
P = 128
R = LN // P               # 16 row tiles per core
KC = D // P               # 4 contraction chunks
CH = 512                  # matmul moving free dim (one PSUM bank)
CPB = LN // CH            # 4 chunks per block
NB = NCORES               # 8 column blocks (one per source core)
GW = 1024                 # stat-group width (2 PSUM banks -> 4 pipeline slots)
GPB = LN // GW            # 2 stat groups per block
NG = NB * GPB             # 16 stat groups per row tile
SQS = SCALE ** 0.5        # sqrt(scale), folded into both operands

F32 = mybir.dt.float32
BF16 = mybir.dt.bfloat16


def build():
    nc = bacc.Bacc(None, target_bir_lowering=False, debug=False, num_devices=NCORES)

    img_ext = nc.dram_tensor("image_features", [LN, D], F32, kind="ExternalInput")
    txt_ext = nc.dram_tensor("text_features", [LN, D], F32, kind="ExternalInput")
    out_ext = nc.dram_tensor("out", [P, 4], F32, kind="ExternalOutput")

    with tile.TileContext(nc) as tc:
        with (
            tc.tile_pool(name="dram", bufs=1, space="DRAM") as dram,
            tc.tile_pool(name="const", bufs=1) as const,
            tc.tile_pool(name="persist", bufs=1) as persist,
            tc.tile_pool(name="stats", bufs=1) as stats,
        ):
            itb = dram.tile([D, LN], BF16)
            ttb = dram.tile([D, LN], BF16)
            itg = dram.tile([NCORES * D, LN], BF16, addr_space="Shared")
            ttg = dram.tile([NCORES * D, LN], BF16, addr_space="Shared")

            ident = const.tile([P, P], F32)
            make_identity(nc, ident)

            # persistent D-major bf16 shards: [p=d%128, dk, i]
            imgT = persist.tile([P, KC, LN], BF16)
            txtT = persist.tile([P, KC, LN], BF16)

            # per (row-tile, group) stats, col = r * NG + g; mx holds -max
            mx0 = stats.tile([P, R * NG], F32)
            ss0 = stats.tile([P, R * NG], F32)
            mx1 = stats.tile([P, R * NG], F32)
            ss1 = stats.tile([P, R * NG], F32)
            diag_pp = stats.tile([P, 1], F32)
            lse0_pp = stats.tile([P, 1], F32)
            lse1_pp = stats.tile([P, 1], F32)

            # ---------------- setup: load, diag, transpose, gather ----------
            with (
                tc.tile_pool(name="setup", bufs=1) as setup,
                tc.tile_pool(name="tpsum", bufs=2, space="PSUM") as tpsum,
            ):
                img_sb = setup.tile([P, R, D], F32)
                txt_sb = setup.tile([P, R, D], F32)
                nc.sync.dma_start(img_sb[:], img_ext[:].rearrange("(r p) d -> p r d", p=P))
                nc.sync.dma_start(txt_sb[:], txt_ext[:].rearrange("(r p) d -> p r d", p=P))

                # text first so its AllGather is issued as early as possible
                for src, dstT in ((txt_sb, txtT), (img_sb, imgT)):
                    for r in range(R):
                        tp = tpsum.tile([P, KC, P], F32, name="tp")
                        for dk in range(KC):
                            nc.tensor.transpose(
                                tp[:, dk, :],
                                src[:, r, dk * P:(dk + 1) * P],
                                ident[:],
                            )
                        nc.vector.tensor_scalar_mul(
                            dstT[:, :, r * P:(r + 1) * P], tp[:], SQS
                        )
                    if dstT is txtT:
                        nc.sync.dma_start(
                            ttb[:].rearrange("(dk p) i -> p dk i", p=P), txtT[:]
                        )
                        nc.gpsimd.collective_compute(
                            "AllGather",
                            mybir.AluOpType.bypass,
                            replica_groups=[list(range(NCORES))],
                            ins=[ttb[:].opt()],
                            outs=[ttg[:].opt()],
                        )
                    else:
                        nc.sync.dma_start(
                            itb[:].rearrange("(dk p) i -> p dk i", p=P), imgT[:]
                        )
                        nc.gpsimd.collective_compute(
                            "AllGather",
                            mybir.AluOpType.bypass,
                            replica_groups=[list(range(NCORES))],
                            ins=[itb[:].opt()],
                            outs=[itg[:].opt()],
                        )

                # diag partial: sum_d img[i, d] * txt[i, d]  (unscaled, fp32)
                dtmp = setup.tile([P, R, D], F32)
                nc.vector.tensor_mul(dtmp[:], img_sb[:], txt_sb[:])
                dsum = setup.tile([P, R], F32)
                nc.vector.reduce_sum(dsum[:], dtmp[:], axis=mybir.AxisListType.X)
                nc.vector.reduce_sum(diag_pp[:], dsum[:], axis=mybir.AxisListType.X)

            # ---------------- main passes ----------------------------------
            with (
                tc.tile_pool(name="stream", bufs=2) as stream,
                tc.tile_pool(name="mpsum", bufs=4, space="PSUM") as mpsum,
                tc.tile_pool(name="scratch", bufs=4) as scratch,
                tc.tile_pool(name="small", bufs=4) as small,
            ):
                for lhsT, gsrc, mx, ss, lse_pp in (
                    (imgT, ttg, mx0, ss0, lse0_pp),
                    (txtT, itg, mx1, ss1, lse1_pp),
                ):
                    for b in range(NB):
                        rhs = stream.tile([P, KC, LN], BF16, name="rhs", tag="rhs")
                        nc.sync.dma_start(
                            rhs[:],
                            gsrc[b * D:(b + 1) * D, :].rearrange(
                                "(dk p) j -> p dk j", p=P
                            ),
                        )
                        for r in range(R):
                            for h in range(GPB):
                                pt = mpsum.tile([P, GW], F32, name="pt", tag="pt")
                                for c in range(GW // CH):
                                    cc = h * (GW // CH) + c
                                    for k in range(KC):
                                        nc.tensor.matmul(
                                            pt[:, c * CH:(c + 1) * CH],
                                            lhsT[:, k, r * P:(r + 1) * P],
                                            rhs[:, k, cc * CH:(cc + 1) * CH],
                                            start=(k == 0),
                                            stop=(k == KC - 1),
                                        )
                                col = r * NG + b * GPB + h
                                # nmx = -max(psum); psum already holds SCALE*logits
                                nc.vector.reduce_max(
                                    mx[:, col:col + 1],
                                    pt[:],
                                    axis=mybir.AxisListType.X,
                                    negate=True,
                                )
                                esc = scratch.tile([P, GW], BF16, name="esc", tag="esc")
                                nc.scalar.activation(
                                    esc[:],
                                    pt[:],
                                    mybir.ActivationFunctionType.Exp,
                                    bias=mx[:, col:col + 1],
                                    accum_out=ss[:, col:col + 1],
                                )

                    # epilogue: merge the NG group stats per row tile
                    nm_all = small.tile([P, R], F32, name="nm_all", tag="nm_all")
                    st_all = small.tile([P, R], F32, name="st_all", tag="st_all")
                    for r in range(R):
                        sl = slice(r * NG, (r + 1) * NG)
                        nc.vector.tensor_reduce(
                            nm_all[:, r:r + 1], mx[:, sl],
                            axis=mybir.AxisListType.X, op=mybir.AluOpType.min,
                        )
                        dev = small.tile([P, NG], F32, name="dev", tag="dev")
                        nc.vector.tensor_scalar_sub(
                            dev[:], mx[:, sl], nm_all[:, r:r + 1]
                        )
                        w = small.tile([P, NG], F32, name="w", tag="w")
                        nc.scalar.activation(
                            w[:], dev[:], mybir.ActivationFunctionType.Exp, scale=-1.0
                        )
                        sw = small.tile([P, NG], F32, name="sw", tag="sw")
                        nc.vector.tensor_mul(sw[:], ss[:, sl], w[:])
                        nc.vector.reduce_sum(
                            st_all[:, r:r + 1], sw[:], axis=mybir.AxisListType.X
                        )
                    lgs = small.tile([P, R], F32, name="lgs", tag="lgs")
                    nc.scalar.activation(
                        lgs[:], st_all[:], mybir.ActivationFunctionType.Ln
                    )
                    lse_all = small.tile([P, R], F32, name="lse_all", tag="lse_all")
                    nc.vector.scalar_tensor_tensor(
                        lse_all[:],
                        nm_all[:],
                        -1.0,
                        lgs[:],
                        op0=mybir.AluOpType.mult,
                        op1=mybir.AluOpType.add,
                    )
                    nc.vector.reduce_sum(
                        lse_pp[:], lse_all[:], axis=mybir.AxisListType.X
                    )

                out_sb = small.tile([P, 4], F32, name="out_sb", tag="out_sb")
                nc.vector.memset(out_sb[:], 0.0)
                nc.vector.tensor_copy(out_sb[:, 0:1], lse0_pp[:])
                nc.vector.tensor_copy(out_sb[:, 1:2], lse1_pp[:])
                nc.vector.tensor_copy(out_sb[:, 2:3], diag_pp[:])
                nc.sync.dma_start(out_ext[:], out_sb[:])

    nc.compile()
    return nc


_NC_CACHE = None


def _get_nc():
    global _NC_CACHE
    if _NC_CACHE is None:
        _NC_CACHE = build()
    return _NC_CACHE


def kernel(image_features: np.ndarray, text_features: np.ndarray) -> np.ndarray:
    img = np.ascontiguousarray(np.asarray(image_features, dtype=np.float32))
    txt = np.ascontiguousarray(np.asarray(text_features, dtype=np.float32))
    assert img.shape == (N, D) and txt.shape == (N, D)

    nc = _get_nc()
    in_maps = [
        {
            "image_features": img[i * LN:(i + 1) * LN],
            "text_features": txt[i * LN:(i + 1) * LN],
        }
        for i in range(NCORES)
    ]
    res = run_bass_kernel_spmd(nc, in_maps, core_ids=list(range(NCORES)))

    lse_img = 0.0
    lse_txt = 0.0
    diag = 0.0
    for om in res.results:
        part = om["out"].astype(np.float64)
        lse_img += part[:, 0].sum()
        lse_txt += part[:, 1].sum()
        diag += part[:, 2].sum()

    loss = (lse_img + lse_txt - 2.0 * SCALE * diag) / (2.0 * N)
    return np.float32(loss)


if __name__ == "__main__":
    rng = np.random.default_rng(0)
    a = rng.standard_normal((N, D)).astype(np.float32)
    b = rng.standard_normal((N, D)).astype(np.float32)
    print("loss:", kernel(a, b))


# revision 10
# speedup vs baseline: 1.3108x; 1.0364x over previous
"""CLIP contrastive loss on 8 Trainium2 NeuronCores.

Strategy (data parallel over rows, per the sharding hint):
  - Both feature matrices are row-sharded 8 x [2048, 512].
  - Each core PE-transposes its shards to D-major (cast to bf16), then
    AllGathers both transposed matrices (bf16 halves the wire bytes).
  - Pass 1 (image rows): L = img_shard @ txt_all^T computed in [128, 2048]
    PSUM tiles; per tile: VectorE rowmax, then one fused ScalarE
    activation exp(SCALE*x - SCALE*max) with free-dim sum accumulation.
    Per-row-block group stats are merged into a logsumexp at the end.
  - Pass 2 (text rows): same with roles swapped.
  - diag terms <img_i, txt_i> computed in fp32 on VectorE.
  - Each core outputs [128, 4] partial sums; the host reduces them to the
    scalar loss. All O(N^2) work happens on-device.
"""
import sys

if "/opt/trn_rl_repo" not in sys.path:
    sys.path.insert(0, "/opt/trn_rl_repo")

import numpy as np

from concourse import bacc, bass, mybir, tile
from concourse.bass_utils import run_bass_kernel_spmd
from concourse.masks import make_identity

SCALE = 1.0 / 0.07
N = 16384
D = 512
NCORES = 8
LN = N // NCORES          # 2048 local rows
P = 128
R = LN // P               # 16 row tiles per core
KC = D // P               # 4 contraction chunks
CH = 512                  # matmul moving free dim (one PSUM bank)
CPB = LN // CH            # 4 chunks per block
NB = NCORES               # 8 column blocks (one per source core)
GW = 1024                 # stat-group width (2 PSUM banks -> 4 pipeline slots)
GPB = LN // GW            # 2 stat groups per block
NG = NB * GPB             # 16 stat groups per row tile
SQS = SCALE ** 0.5        # sqrt(scale), folded into both operands

F32 = mybir.dt.float32
BF16 = mybir.dt.bfloat16


def build():
    nc = bacc.Bacc(None, target_bir_lowering=False, debug=False, num_devices=NCORES)

    img_ext = nc.dram_tensor("image_features", [LN, D], F32, kind="ExternalInput")
    txt_ext = nc.dram_tensor("text_features", [LN, D], F32, kind="ExternalInput")
    out_ext = nc.dram_tensor("out", [P, 4], F32, kind="ExternalOutput")

    with tile.TileContext(nc) as tc:
        with (
            tc.tile_pool(name="dram", bufs=1, space="DRAM") as dram,
            tc.tile_pool(name="const", bufs=1) as const,
            tc.tile_pool(name="persist", bufs=1) as persist,
            tc.tile_pool(name="stats", bufs=1) as stats,
        ):
            itb = dram.tile([D, LN], BF16)
            ttb = dram.tile([D, LN], BF16)
            itg = dram.tile([NCORES * D, LN], BF16, addr_space="Shared")
            ttg = dram.tile([NCORES * D, LN], BF16, addr_space="Shared")

            ident = const.tile([P, P], F32)
            make_identity(nc, ident)

            # persistent D-major bf16 shards: [p=d%128, dk, i]
            imgT = persist.tile([P, KC, LN], BF16)
            txtT = persist.tile([P, KC, LN], BF16)

            # per (row-tile, group) stats, col = r * NG + g; mx holds -max
            mx0 = stats.tile([P, R * NG], F32)
            ss0 = stats.tile([P, R * NG], F32)
            mx1 = stats.tile([P, R * NG], F32)
            ss1 = stats.tile([P, R * NG], F32)
            diag_pp = stats.tile([P, 1], F32)
            lse0_pp = stats.tile([P, 1], F32)
            lse1_pp = stats.tile([P, 1], F32)

            # ---------------- setup: load, diag, transpose, gather ----------
            with (
                tc.tile_pool(name="setup", bufs=1) as setup,
                tc.tile_pool(name="tpsum", bufs=2, space="PSUM") as tpsum,
            ):
                img_sb = setup.tile([P, R, D], F32)
                txt_sb = setup.tile([P, R, D], F32)
                nc.sync.dma_start(txt_sb[:], txt_ext[:].rearrange("(r p) d -> p r d", p=P))
                nc.scalar.dma_start(img_sb[:], img_ext[:].rearrange("(r p) d -> p r d", p=P))

                # text first so its AllGather is issued as early as possible
                for src, dstT in ((txt_sb, txtT), (img_sb, imgT)):
                    for r in range(R):
                        tp = tpsum.tile([P, KC, P], F32, name="tp")
                        for dk in range(KC):
                            nc.tensor.transpose(
                                tp[:, dk, :],
                                src[:, r, dk * P:(dk + 1) * P],
                                ident[:],
                            )
                        nc.vector.tensor_scalar_mul(
                            dstT[:, :, r * P:(r + 1) * P], tp[:], SQS
                        )
                    if dstT is txtT:
                        nc.sync.dma_start(
                            ttb[:].rearrange("(dk p) i -> p dk i", p=P), txtT[:]
                        )
                        nc.gpsimd.collective_compute(
                            "AllGather",
                            mybir.AluOpType.bypass,
                            replica_groups=[list(range(NCORES))],
                            ins=[ttb[:].opt()],
                            outs=[ttg[:].opt()],
                        )
                    else:
                        nc.sync.dma_start(
                            itb[:].rearrange("(dk p) i -> p dk i", p=P), imgT[:]
                        )
                        nc.gpsimd.collective_compute(
                            "AllGather",
                            mybir.AluOpType.bypass,
                            replica_groups=[list(range(NCORES))],
                            ins=[itb[:].opt()],
                            outs=[itg[:].opt()],
                        )

                # diag partial: sum_d img[i, d] * txt[i, d]  (unscaled, fp32)
                dtmp = setup.tile([P, R, D], F32)
                nc.vector.tensor_mul(dtmp[:], img_sb[:], txt_sb[:])
                dsum = setup.tile([P, R], F32)
                nc.vector.reduce_sum(dsum[:], dtmp[:], axis=mybir.AxisListType.X)
                nc.vector.reduce_sum(diag_pp[:], dsum[:], axis=mybir.AxisListType.X)

            # ---------------- main passes ----------------------------------
            with (
                tc.tile_pool(name="stream", bufs=2) as stream,
                tc.tile_pool(name="mpsum", bufs=4, space="PSUM") as mpsum,
                tc.tile_pool(name="scratch", bufs=4) as scratch,
                tc.tile_pool(name="small", bufs=4) as small,
            ):
                # rank of this core: s=0 uses the SBUF-resident own block while
                # the AllGather is still in flight; s>0 reads block (rank+s)%8
                # from the gathered buffer.
                rank = nc.sync.snap(
                    nc.sync.cc_rank(replica_groups=[list(range(NCORES))]),
                    min_val=0,
                    max_val=NCORES - 1,
                )
                for lhsT, gsrc, mx, ss, lse_pp, own in (
                    (imgT, ttg, mx0, ss0, lse0_pp, txtT),
                    (txtT, itg, mx1, ss1, lse1_pp, imgT),
                ):
                    for b in range(NB):
                        if b == 0:
                            rhs = own
                        else:
                            rhs = stream.tile([P, KC, LN], BF16, name="rhs", tag="rhs")
                            bb = (rank + b) % NCORES
                            nc.sync.dma_start(
                                rhs[:],
                                gsrc[bass.ds(bb * D, D), :].rearrange(
                                    "(dk p) j -> p dk j", p=P
                                ),
                            )
                        for r in range(R):
                            for h in range(GPB):
                                pt = mpsum.tile([P, GW], F32, name="pt", tag="pt")
                                for c in range(GW // CH):
                                    cc = h * (GW // CH) + c
                                    for k in range(KC):
                                        nc.tensor.matmul(
                                            pt[:, c * CH:(c + 1) * CH],
                                            lhsT[:, k, r * P:(r + 1) * P],
                                            rhs[:, k, cc * CH:(cc + 1) * CH],
                                            start=(k == 0),
                                            stop=(k == KC - 1),
                                        )
                                col = r * NG + b * GPB + h
                                # nmx = -max(psum); psum already holds SCALE*logits
                                nc.vector.reduce_max(
                                    mx[:, col:col + 1],
                                    pt[:],
                                    axis=mybir.AxisListType.X,
                                    negate=True,
                                )
                                esc = scratch.tile([P, GW], BF16, name="esc", tag="esc")
                                nc.scalar.activation(
                                    esc[:],
                                    pt[:],
                                    mybir.ActivationFunctionType.Exp,
                                    bias=mx[:, col:col + 1],
                                    accum_out=ss[:, col:col + 1],
                                )

                    # epilogue: merge the NG group stats per row tile
                    nm_all = small.tile([P, R], F32, name="nm_all", tag="nm_all")
                    st_all = small.tile([P, R], F32, name="st_all", tag="st_all")
                    for r in range(R):
                        sl = slice(r * NG, (r + 1) * NG)
                        nc.vector.tensor_reduce(
                            nm_all[:, r:r + 1], mx[:, sl],
                            axis=mybir.AxisListType.X, op=mybir.AluOpType.min,
                        )
                        dev = small.tile([P, NG], F32, name="dev", tag="dev")
                        nc.vector.tensor_scalar_sub(
                            dev[:], mx[:, sl], nm_all[:, r:r + 1]
                        )
                        w = small.tile([P, NG], F32, name="w", tag="w")
                        nc.scalar.activation(
                            w[:], dev[:], mybir.ActivationFunctionType.Exp, scale=-1.0
                        )
                        sw = small.tile([P, NG], F32, name="sw", tag="sw")
                        nc.vector.tensor_mul(sw[:], ss[:, sl], w[:])
                        nc.vector.reduce_sum(
                            st_all[:, r:r + 1], sw[:], axis=mybir.AxisListType.X
                        )
                    lgs = small.tile([P, R], F32, name="lgs", tag="lgs")
                    nc.scalar.activation(
                        lgs[:], st_all[:], mybir.ActivationFunctionType.Ln
                    )
                    lse_all = small.tile([P, R], F32, name="lse_all", tag="lse_all")
                    nc.vector.scalar_tensor_tensor(
                        lse_all[:],
                        nm_all[:],
                        -1.0,
                        lgs[:],
                        op0=mybir.AluOpType.mult,
                        op1=mybir.AluOpType.add,
                    )
                    nc.vector.reduce_sum(
                        lse_pp[:], lse_all[:], axis=mybir.AxisListType.X
                    )

                out_sb = small.tile([P, 4], F32, name="out_sb", tag="out_sb")
                nc.vector.memset(out_sb[:], 0.0)
                nc.vector.tensor_copy(out_sb[:, 0:1], lse0_pp[:])
                nc.vector.tensor_copy(out_sb[:, 1:2], lse1_pp[:])
                nc.vector.tensor_copy(out_sb[:, 2:3], diag_pp[:])
                nc.sync.dma_start(out_ext[:], out_sb[:])

    nc.compile()
    return nc


_NC_CACHE = None


def _get_nc():
    global _NC_CACHE
    if _NC_CACHE is None:
        _NC_CACHE = build()
    return _NC_CACHE


def kernel(image_features: np.ndarray, text_features: np.ndarray) -> np.ndarray:
    img = np.ascontiguousarray(np.asarray(image_features, dtype=np.float32))
    txt = np.ascontiguousarray(np.asarray(text_features, dtype=np.float32))
    assert img.shape == (N, D) and txt.shape == (N, D)

    nc = _get_nc()
    in_maps = [
        {
            "image_features": img[i * LN:(i + 1) * LN],
            "text_features": txt[i * LN:(i + 1) * LN],
        }
        for i in range(NCORES)
    ]
    res = run_bass_kernel_spmd(nc, in_maps, core_ids=list(range(NCORES)))

    lse_img = 0.0
    lse_txt = 0.0
    diag = 0.0
    for om in res.results:
        part = om["out"].astype(np.float64)
        lse_img += part[:, 0].sum()
        lse_txt += part[:, 1].sum()
        diag += part[:, 2].sum()

    loss = (lse_img + lse_txt - 2.0 * SCALE * diag) / (2.0 * N)
    return np.float32(loss)


if __name__ == "__main__":
    rng = np.random.default_rng(0)
    a = rng.standard_normal((N, D)).astype(np.float32)
    b = rng.standard_normal((N, D)).astype(np.float32)
    print("loss:", kernel(a, b))


# revision 12
# speedup vs baseline: 1.3806x; 1.0533x over previous
"""CLIP contrastive loss on 8 Trainium2 NeuronCores.

Strategy (data parallel over rows, per the sharding hint):
  - Both feature matrices are row-sharded 8 x [2048, 512].
  - Each core PE-transposes its shards to D-major (cast to bf16), then
    AllGathers both transposed matrices (bf16 halves the wire bytes).
  - Pass 1 (image rows): L = img_shard @ txt_all^T computed in [128, 2048]
    PSUM tiles; per tile: VectorE rowmax, then one fused ScalarE
    activation exp(SCALE*x - SCALE*max) with free-dim sum accumulation.
    Per-row-block group stats are merged into a logsumexp at the end.
  - Pass 2 (text rows): same with roles swapped.
  - diag terms <img_i, txt_i> computed in fp32 on VectorE.
  - Each core outputs [128, 4] partial sums; the host reduces them to the
    scalar loss. All O(N^2) work happens on-device.
"""
import sys

if "/opt/trn_rl_repo" not in sys.path:
    sys.path.insert(0, "/opt/trn_rl_repo")

import numpy as np

from concourse import bacc, bass, mybir, tile
from concourse.bass_utils import run_bass_kernel_spmd
from concourse.masks import make_identity

SCALE = 1.0 / 0.07
N = 16384
D = 512
NCORES = 8
LN = N // NCORES          # 2048 local rows
P = 128
R = LN // P               # 16 row tiles per core
KC = D // P               # 4 contraction chunks
CH = 512                  # matmul moving free dim (one PSUM bank)
CPB = LN // CH            # 4 chunks per block
NB = NCORES               # 8 column blocks (one per source core)
GW = 1024                 # stat-group width (2 PSUM banks -> 4 pipeline slots)
GPB = LN // GW            # 2 stat groups per block
NG = NB * GPB             # 16 stat groups per row tile
SQS = SCALE ** 0.5        # sqrt(scale), folded into both operands

F32 = mybir.dt.float32
BF16 = mybir.dt.bfloat16


def build():
    nc = bacc.Bacc(None, target_bir_lowering=False, debug=False, num_devices=NCORES)

    img_ext = nc.dram_tensor("image_features", [LN, D], F32, kind="ExternalInput")
    txt_ext = nc.dram_tensor("text_features", [LN, D], F32, kind="ExternalInput")
    out_ext = nc.dram_tensor("out", [P, 4], F32, kind="ExternalOutput")

    with tile.TileContext(nc) as tc:
        with (
            tc.tile_pool(name="dram", bufs=1, space="DRAM") as dram,
            tc.tile_pool(name="const", bufs=1) as const,
            tc.tile_pool(name="persist", bufs=1) as persist,
            tc.tile_pool(name="stats", bufs=1) as stats,
        ):
            itb = dram.tile([D, LN], BF16)
            ttb = dram.tile([D, LN], BF16)
            itg = dram.tile([NCORES * D, LN], BF16, addr_space="Shared")
            ttg = dram.tile([NCORES * D, LN], BF16, addr_space="Shared")

            ident = const.tile([P, P], F32)
            make_identity(nc, ident)

            # persistent D-major bf16 shards: [p=d%128, dk, i]
            imgT = persist.tile([P, KC, LN], BF16)
            txtT = persist.tile([P, KC, LN], BF16)

            # per (row-tile, group) stats, col = r * NG + g; mx holds -max
            mx0 = stats.tile([P, R * NG], F32)
            ss0 = stats.tile([P, R * NG], F32)
            mx1 = stats.tile([P, R * NG], F32)
            ss1 = stats.tile([P, R * NG], F32)
            diag_pp = stats.tile([P, 1], F32)
            lse0_pp = stats.tile([P, 1], F32)
            lse1_pp = stats.tile([P, 1], F32)

            # ---------------- setup: load, diag, transpose, gather ----------
            with (
                tc.tile_pool(name="setup", bufs=1) as setup,
                tc.tile_pool(name="tpsum", bufs=2, space="PSUM") as tpsum,
            ):
                img_sb = setup.tile([P, R, D], F32)
                txt_sb = setup.tile([P, R, D], F32)
                RQ = R // 4
                for q in range(4):
                    nc.sync.dma_start(
                        txt_sb[:, q * RQ:(q + 1) * RQ, :],
                        txt_ext[q * RQ * P:(q + 1) * RQ * P, :].rearrange(
                            "(r p) d -> p r d", p=P
                        ),
                    )
                nc.scalar.dma_start(img_sb[:], img_ext[:].rearrange("(r p) d -> p r d", p=P))

                # text first so its AllGather is issued as early as possible
                for src, dstT in ((txt_sb, txtT), (img_sb, imgT)):
                    for r in range(R):
                        tp = tpsum.tile([P, KC, P], F32, name="tp")
                        for dk in range(KC):
                            nc.tensor.transpose(
                                tp[:, dk, :],
                                src[:, r, dk * P:(dk + 1) * P],
                                ident[:],
                            )
                        nc.vector.tensor_scalar_mul(
                            dstT[:, :, r * P:(r + 1) * P], tp[:], SQS
                        )
                    if dstT is txtT:
                        nc.sync.dma_start(
                            ttb[:].rearrange("(dk p) i -> p dk i", p=P), txtT[:]
                        )
                        nc.gpsimd.collective_compute(
                            "AllGather",
                            mybir.AluOpType.bypass,
                            replica_groups=[list(range(NCORES))],
                            ins=[ttb[:].opt()],
                            outs=[ttg[:].opt()],
                        )
                    else:
                        nc.sync.dma_start(
                            itb[:].rearrange("(dk p) i -> p dk i", p=P), imgT[:]
                        )
                        nc.gpsimd.collective_compute(
                            "AllGather",
                            mybir.AluOpType.bypass,
                            replica_groups=[list(range(NCORES))],
                            ins=[itb[:].opt()],
                            outs=[itg[:].opt()],
                        )

                # diag partial: sum_d img[i, d] * txt[i, d]  (unscaled, fp32)
                dtmp = setup.tile([P, R, D], F32)
                nc.vector.tensor_mul(dtmp[:], img_sb[:], txt_sb[:])
                dsum = setup.tile([P, R], F32)
                nc.vector.reduce_sum(dsum[:], dtmp[:], axis=mybir.AxisListType.X)
                nc.vector.reduce_sum(diag_pp[:], dsum[:], axis=mybir.AxisListType.X)

            # ---------------- main passes ----------------------------------
            with (
                tc.tile_pool(name="stream", bufs=2) as stream,
                tc.tile_pool(name="mpsum", bufs=4, space="PSUM") as mpsum,
                tc.tile_pool(name="scratch", bufs=4) as scratch,
                tc.tile_pool(name="small", bufs=4) as small,
            ):
                # rank of this core: block s=0 of each pass uses the
                # SBUF-resident own shard while the AllGathers are in flight;
                # s>0 reads block (rank+s)%8 from the gathered buffer.
                rank = nc.sync.snap(
                    nc.sync.cc_rank(replica_groups=[list(range(NCORES))]),
                    min_val=0,
                    max_val=NCORES - 1,
                )
                cfgs = [
                    (imgT, ttg, mx0, ss0, lse0_pp, txtT),
                    (txtT, itg, mx1, ss1, lse1_pp, imgT),
                ]

                def emit_block(pi, s):
                    lhsT, gsrc, mx, ss, _, own = cfgs[pi]
                    if s == 0:
                        rhs = own
                    else:
                        rhs = stream.tile([P, KC, LN], BF16, name="rhs", tag="rhs")
                        bb = (rank + s) % NCORES
                        nc.sync.dma_start(
                            rhs[:],
                            gsrc[bass.ds(bb * D, D), :].rearrange(
                                "(dk p) j -> p dk j", p=P
                            ),
                        )
                    for r in range(R):
                        for h in range(GPB):
                            pt = mpsum.tile([P, GW], F32, name="pt", tag="pt")
                            for c in range(GW // CH):
                                cc = h * (GW // CH) + c
                                for k in range(KC):
                                    nc.tensor.matmul(
                                        pt[:, c * CH:(c + 1) * CH],
                                        lhsT[:, k, r * P:(r + 1) * P],
                                        rhs[:, k, cc * CH:(cc + 1) * CH],
                                        start=(k == 0),
                                        stop=(k == KC - 1),
                                    )
                            col = r * NG + s * GPB + h
                            # nmx = -max(psum); psum already holds SCALE*logits
                            nc.vector.reduce_max(
                                mx[:, col:col + 1],
                                pt[:],
                                axis=mybir.AxisListType.X,
                                negate=True,
                            )
                            esc = scratch.tile([P, GW], BF16, name="esc", tag="esc")
                            nc.scalar.activation(
                                esc[:],
                                pt[:],
                                mybir.ActivationFunctionType.Exp,
                                bias=mx[:, col:col + 1],
                                accum_out=ss[:, col:col + 1],
                            )

                def emit_epilogue(pi):
                    _, _, mx, ss, lse_pp, _ = cfgs[pi]
                    nm_all = small.tile([P, R], F32, name="nm_all", tag="nm_all")
                    st_all = small.tile([P, R], F32, name="st_all", tag="st_all")
                    for r in range(R):
                        sl = slice(r * NG, (r + 1) * NG)
                        nc.vector.tensor_reduce(
                            nm_all[:, r:r + 1], mx[:, sl],
                            axis=mybir.AxisListType.X, op=mybir.AluOpType.min,
                        )
                        dev = small.tile([P, NG], F32, name="dev", tag="dev")
                        nc.vector.tensor_scalar_sub(
                            dev[:], mx[:, sl], nm_all[:, r:r + 1]
                        )
                        w = small.tile([P, NG], F32, name="w", tag="w")
                        nc.scalar.activation(
                            w[:], dev[:], mybir.ActivationFunctionType.Exp, scale=-1.0
                        )
                        sw = small.tile([P, NG], F32, name="sw", tag="sw")
                        nc.vector.tensor_mul(sw[:], ss[:, sl], w[:])
                        nc.vector.reduce_sum(
                            st_all[:, r:r + 1], sw[:], axis=mybir.AxisListType.X
                        )
                    lgs = small.tile([P, R], F32, name="lgs", tag="lgs")
                    nc.scalar.activation(
                        lgs[:], st_all[:], mybir.ActivationFunctionType.Ln
                    )
                    lse_all = small.tile([P, R], F32, name="lse_all", tag="lse_all")
                    nc.vector.scalar_tensor_tensor(
                        lse_all[:],
                        nm_all[:],
                        -1.0,
                        lgs[:],
                        op0=mybir.AluOpType.mult,
                        op1=mybir.AluOpType.add,
                    )
                    nc.vector.reduce_sum(
                        lse_pp[:], lse_all[:], axis=mybir.AxisListType.X
                    )

                emit_block(0, 0)
                emit_block(1, 0)
                for s in range(1, NB):
                    emit_block(0, s)
                emit_epilogue(0)
                for s in range(1, NB):
                    emit_block(1, s)
                emit_epilogue(1)

                out_sb = small.tile([P, 4], F32, name="out_sb", tag="out_sb")
                nc.vector.memset(out_sb[:], 0.0)
                nc.vector.tensor_copy(out_sb[:, 0:1], lse0_pp[:])
                nc.vector.tensor_copy(out_sb[:, 1:2], lse1_pp[:])
                nc.vector.tensor_copy(out_sb[:, 2:3], diag_pp[:])
                nc.sync.dma_start(out_ext[:], out_sb[:])

    nc.compile()
    return nc


_NC_CACHE = None


def _get_nc():
    global _NC_CACHE
    if _NC_CACHE is None:
        _NC_CACHE = build()
    return _NC_CACHE


def kernel(image_features: np.ndarray, text_features: np.ndarray) -> np.ndarray:
    img = np.ascontiguousarray(np.asarray(image_features, dtype=np.float32))
    txt = np.ascontiguousarray(np.asarray(text_features, dtype=np.float32))
    assert img.shape == (N, D) and txt.shape == (N, D)

    nc = _get_nc()
    in_maps = [
        {
            "image_features": img[i * LN:(i + 1) * LN],
            "text_features": txt[i * LN:(i + 1) * LN],
        }
        for i in range(NCORES)
    ]
    res = run_bass_kernel_spmd(nc, in_maps, core_ids=list(range(NCORES)))

    lse_img = 0.0
    lse_txt = 0.0
    diag = 0.0
    for om in res.results:
        part = om["out"].astype(np.float64)
        lse_img += part[:, 0].sum()
        lse_txt += part[:, 1].sum()
        diag += part[:, 2].sum()

    loss = (lse_img + lse_txt - 2.0 * SCALE * diag) / (2.0 * N)
    return np.float32(loss)


if __name__ == "__main__":
    rng = np.random.default_rng(0)
    a = rng.standard_normal((N, D)).astype(np.float32)
    b = rng.standard_normal((N, D)).astype(np.float32)
    print("loss:", kernel(a, b))


# revision 13
# speedup vs baseline: 2.2805x; 1.6518x over previous
"""CLIP contrastive loss on 8 Trainium2 NeuronCores.

Strategy (data parallel over rows, per the sharding hint):
  - Both feature matrices are row-sharded 8 x [2048, 512].
  - Each core PE-transposes its shards to D-major (cast to bf16), then
    AllGathers both transposed matrices (bf16 halves the wire bytes).
  - Pass 1 (image rows): L = img_shard @ txt_all^T computed in [128, 2048]
    PSUM tiles; per tile: VectorE rowmax, then one fused ScalarE
    activation exp(SCALE*x - SCALE*max) with free-dim sum accumulation.
    Per-row-block group stats are merged into a logsumexp at the end.
  - Pass 2 (text rows): same with roles swapped.
  - diag terms <img_i, txt_i> computed in fp32 on VectorE.
  - Each core outputs [128, 4] partial sums; the host reduces them to the
    scalar loss. All O(N^2) work happens on-device.
"""
import sys

if "/opt/trn_rl_repo" not in sys.path:
    sys.path.insert(0, "/opt/trn_rl_repo")

import numpy as np

from concourse import bacc, bass, mybir, tile
from concourse.bass_utils import run_bass_kernel_spmd
from concourse.masks import make_identity

SCALE = 1.0 / 0.07
N = 16384
D = 512
NCORES = 8
LN = N // NCORES          # 2048 local rows
P = 128
R = LN // P               # 16 row tiles per core
KC = D // P               # 4 contraction chunks
CH = 512                  # matmul moving free dim (one PSUM bank)
CPB = LN // CH            # 4 chunks per block
NB = NCORES               # 8 column blocks (one per source core)
GW = 1024                 # stat-group width (2 PSUM banks -> 4 pipeline slots)
GPB = LN // GW            # 2 stat groups per block
NG = NB * GPB             # 16 stat groups per row tile
SQS = SCALE ** 0.5        # sqrt(scale), folded into both operands

F32 = mybir.dt.float32
BF16 = mybir.dt.bfloat16
FP8 = mybir.dt.float8e4


def build():
    nc = bacc.Bacc(None, target_bir_lowering=False, debug=False, num_devices=NCORES)

    img_ext = nc.dram_tensor("image_features", [LN, D], F32, kind="ExternalInput")
    txt_ext = nc.dram_tensor("text_features", [LN, D], F32, kind="ExternalInput")
    out_ext = nc.dram_tensor("out", [P, 4], F32, kind="ExternalOutput")

    with tile.TileContext(nc) as tc:
        with (
            tc.tile_pool(name="dram", bufs=1, space="DRAM") as dram,
            tc.tile_pool(name="const", bufs=1) as const,
            tc.tile_pool(name="persist", bufs=1) as persist,
            tc.tile_pool(name="stats", bufs=1) as stats,
        ):
            itb = dram.tile([D, LN], FP8)
            ttb = dram.tile([D, LN], FP8)
            itg = dram.tile([NCORES * D, LN], FP8, addr_space="Shared")
            ttg = dram.tile([NCORES * D, LN], FP8, addr_space="Shared")

            ident = const.tile([P, P], F32)
            make_identity(nc, ident)

            # persistent D-major bf16 shards: [p=d%128, dk, i]
            imgT = persist.tile([P, KC, LN], FP8)
            txtT = persist.tile([P, KC, LN], FP8)

            # per (row-tile, group) stats, col = r * NG + g; mx holds -max
            mx0 = stats.tile([P, R * NG], F32)
            ss0 = stats.tile([P, R * NG], F32)
            mx1 = stats.tile([P, R * NG], F32)
            ss1 = stats.tile([P, R * NG], F32)
            diag_pp = stats.tile([P, 1], F32)
            lse0_pp = stats.tile([P, 1], F32)
            lse1_pp = stats.tile([P, 1], F32)

            # ---------------- setup: load, diag, transpose, gather ----------
            with (
                tc.tile_pool(name="setup", bufs=1) as setup,
                tc.tile_pool(name="tpsum", bufs=2, space="PSUM") as tpsum,
            ):
                img_sb = setup.tile([P, R, D], F32)
                txt_sb = setup.tile([P, R, D], F32)
                RQ = R // 4
                for q in range(4):
                    nc.sync.dma_start(
                        txt_sb[:, q * RQ:(q + 1) * RQ, :],
                        txt_ext[q * RQ * P:(q + 1) * RQ * P, :].rearrange(
                            "(r p) d -> p r d", p=P
                        ),
                    )
                nc.scalar.dma_start(img_sb[:], img_ext[:].rearrange("(r p) d -> p r d", p=P))

                # text first so its AllGather is issued as early as possible
                for src, dstT in ((txt_sb, txtT), (img_sb, imgT)):
                    for r in range(R):
                        tp = tpsum.tile([P, KC, P], F32, name="tp")
                        for dk in range(KC):
                            nc.tensor.transpose(
                                tp[:, dk, :],
                                src[:, r, dk * P:(dk + 1) * P],
                                ident[:],
                            )
                        nc.vector.tensor_scalar_mul(
                            dstT[:, :, r * P:(r + 1) * P], tp[:], SQS
                        )
                    if dstT is txtT:
                        nc.sync.dma_start(
                            ttb[:].rearrange("(dk p) i -> p dk i", p=P), txtT[:]
                        )
                        nc.gpsimd.collective_compute(
                            "AllGather",
                            mybir.AluOpType.bypass,
                            replica_groups=[list(range(NCORES))],
                            ins=[ttb[:].opt()],
                            outs=[ttg[:].opt()],
                        )
                    else:
                        nc.sync.dma_start(
                            itb[:].rearrange("(dk p) i -> p dk i", p=P), imgT[:]
                        )
                        nc.gpsimd.collective_compute(
                            "AllGather",
                            mybir.AluOpType.bypass,
                            replica_groups=[list(range(NCORES))],
                            ins=[itb[:].opt()],
                            outs=[itg[:].opt()],
                        )

                # diag partial: sum_d img[i, d] * txt[i, d]  (unscaled, fp32)
                dtmp = setup.tile([P, R, D], F32)
                nc.vector.tensor_mul(dtmp[:], img_sb[:], txt_sb[:])
                dsum = setup.tile([P, R], F32)
                nc.vector.reduce_sum(dsum[:], dtmp[:], axis=mybir.AxisListType.X)
                nc.vector.reduce_sum(diag_pp[:], dsum[:], axis=mybir.AxisListType.X)

            # ---------------- main passes ----------------------------------
            with (
                tc.tile_pool(name="stream", bufs=2) as stream,
                tc.tile_pool(name="mpsum", bufs=4, space="PSUM") as mpsum,
                tc.tile_pool(name="scratch", bufs=4) as scratch,
                tc.tile_pool(name="small", bufs=4) as small,
            ):
                # rank of this core: block s=0 of each pass uses the
                # SBUF-resident own shard while the AllGathers are in flight;
                # s>0 reads block (rank+s)%8 from the gathered buffer.
                rank = nc.sync.snap(
                    nc.sync.cc_rank(replica_groups=[list(range(NCORES))]),
                    min_val=0,
                    max_val=NCORES - 1,
                )
                cfgs = [
                    (imgT, ttg, mx0, ss0, lse0_pp, txtT),
                    (txtT, itg, mx1, ss1, lse1_pp, imgT),
                ]

                def emit_block(pi, s):
                    lhsT, gsrc, mx, ss, _, own = cfgs[pi]
                    if s == 0:
                        rhs = own
                    else:
                        rhs = stream.tile([P, KC, LN], FP8, name="rhs", tag="rhs")
                        bb = (rank + s) % NCORES
                        nc.sync.dma_start(
                            rhs[:],
                            gsrc[bass.ds(bb * D, D), :].rearrange(
                                "(dk p) j -> p dk j", p=P
                            ),
                        )
                    for r in range(R):
                        for h in range(GPB):
                            pt = mpsum.tile([P, GW], F32, name="pt", tag="pt")
                            for c in range(GW // CH):
                                cc = h * (GW // CH) + c
                                for k in range(0, KC, 2):
                                    nc.tensor.matmul(
                                        pt[:, c * CH:(c + 1) * CH],
                                        lhsT[:, k:k + 2, r * P:(r + 1) * P],
                                        rhs[:, k:k + 2, cc * CH:(cc + 1) * CH],
                                        start=(k == 0),
                                        stop=(k == KC - 2),
                                        perf_mode=mybir.MatmulPerfMode.DoubleRow,
                                    )
                            col = r * NG + s * GPB + h
                            # nmx = -max(psum); psum already holds SCALE*logits
                            nc.vector.reduce_max(
                                mx[:, col:col + 1],
                                pt[:],
                                axis=mybir.AxisListType.X,
                                negate=True,
                            )
                            esc = scratch.tile([P, GW], BF16, name="esc", tag="esc")
                            nc.scalar.activation(
                                esc[:],
                                pt[:],
                                mybir.ActivationFunctionType.Exp,
                                bias=mx[:, col:col + 1],
                                accum_out=ss[:, col:col + 1],
                            )

                def emit_epilogue(pi):
                    _, _, mx, ss, lse_pp, _ = cfgs[pi]
                    nm_all = small.tile([P, R], F32, name="nm_all", tag="nm_all")
                    st_all = small.tile([P, R], F32, name="st_all", tag="st_all")
                    for r in range(R):
                        sl = slice(r * NG, (r + 1) * NG)
                        nc.vector.tensor_reduce(
                            nm_all[:, r:r + 1], mx[:, sl],
                            axis=mybir.AxisListType.X, op=mybir.AluOpType.min,
                        )
                        dev = small.tile([P, NG], F32, name="dev", tag="dev")
                        nc.vector.tensor_scalar_sub(
                            dev[:], mx[:, sl], nm_all[:, r:r + 1]
                        )
                        w = small.tile([P, NG], F32, name="w", tag="w")
                        nc.scalar.activation(
                            w[:], dev[:], mybir.ActivationFunctionType.Exp, scale=-1.0
                        )
                        sw = small.tile([P, NG], F32, name="sw", tag="sw")
                        nc.vector.tensor_mul(sw[:], ss[:, sl], w[:])
                        nc.vector.reduce_sum(
                            st_all[:, r:r + 1], sw[:], axis=mybir.AxisListType.X
                        )
                    lgs = small.tile([P, R], F32, name="lgs", tag="lgs")
                    nc.scalar.activation(
                        lgs[:], st_all[:], mybir.ActivationFunctionType.Ln
                    )
                    lse_all = small.tile([P, R], F32, name="lse_all", tag="lse_all")
                    nc.vector.scalar_tensor_tensor(
                        lse_all[:],
                        nm_all[:],
                        -1.0,
                        lgs[:],
                        op0=mybir.AluOpType.mult,
                        op1=mybir.AluOpType.add,
                    )
                    nc.vector.reduce_sum(
                        lse_pp[:], lse_all[:], axis=mybir.AxisListType.X
                    )

                emit_block(0, 0)
                emit_block(1, 0)
                for s in range(1, NB):
                    emit_block(0, s)
                emit_epilogue(0)
                for s in range(1, NB):
                    emit_block(1, s)
                emit_epilogue(1)

                out_sb = small.tile([P, 4], F32, name="out_sb", tag="out_sb")
                nc.vector.memset(out_sb[:], 0.0)
                nc.vector.tensor_copy(out_sb[:, 0:1], lse0_pp[:])
                nc.vector.tensor_copy(out_sb[:, 1:2], lse1_pp[:])
                nc.vector.tensor_copy(out_sb[:, 2:3], diag_pp[:])
                nc.sync.dma_start(out_ext[:], out_sb[:])

    nc.compile()
    return nc


_NC_CACHE = None


def _get_nc():
    global _NC_CACHE
    if _NC_CACHE is None:
        _NC_CACHE = build()
    return _NC_CACHE


def kernel(image_features: np.ndarray, text_features: np.ndarray) -> np.ndarray:
    img = np.ascontiguousarray(np.asarray(image_features, dtype=np.float32))
    txt = np.ascontiguousarray(np.asarray(text_features, dtype=np.float32))
    assert img.shape == (N, D) and txt.shape == (N, D)

    nc = _get_nc()
    in_maps = [
        {
            "image_features": img[i * LN:(i + 1) * LN],
            "text_features": txt[i * LN:(i + 1) * LN],
        }
        for i in range(NCORES)
    ]
    res = run_bass_kernel_spmd(nc, in_maps, core_ids=list(range(NCORES)))

    lse_img = 0.0
    lse_txt = 0.0
    diag = 0.0
    for om in res.results:
        part = om["out"].astype(np.float64)
        lse_img += part[:, 0].sum()
        lse_txt += part[:, 1].sum()
        diag += part[:, 2].sum()

    loss = (lse_img + lse_txt - 2.0 * SCALE * diag) / (2.0 * N)
    return np.float32(loss)


if __name__ == "__main__":
    rng = np.random.default_rng(0)
    a = rng.standard_normal((N, D)).astype(np.float32)
    b = rng.standard_normal((N, D)).astype(np.float32)
    print("loss:", kernel(a, b))
